# revision 13
# baseline (speedup 1.0000x reference)
"""PointerNetwork TRN2 Bass kernel.

Data parallel: batch 256 split across 8 cores (32 each). Layouts:

  LSTM (enc+dec):  gates (32b x 2048) PSUM; stationary = h.T chunks
                   (128h, 32b), moving = Whh.T (128h, 512). Input-side
                   matmul folded into a K=2 outer product with [x_t; 1]
                   (F_IN=1), bias included.
  w1enc:           (128h' x [4c][32b][128s]) built incrementally during
                   the encoder (W1 @ h_t per step, fp32-exact).
  attention:       DVE stt broadcast-add of q.T onto w1enc, ACT tanh in
                   place, PE v-dot via zero-padded lhsT (128,8) into one
                   (8,512) PSUM accumulation group (32 matmuls, 1 group).
  softmax/argmax:  u8 (8,512) -> DMA reshape -> u (32b,128s) -> DRAM
                   scratch; max/max_index + iota-compare gather for the
                   pointer feedback; log-softmax normalization deferred
                   to one post-loop pass (exp/ln table set loaded once).

Precision (cfg): fp32 matmuls are exact on PE but stream 4 cycles/row;
fp32r (1s8e11m) streams 1 cycle/row at N>=256 but rounds inputs (RNE,
verified on HW). h-split carries h as an fp32r hi+lo pair => exact h
through the recurrence at 2 accumulation passes.

All matmul/transpose operands are produced by DVE/ACT instructions, not
raw DMA: self-loading 4-byte matmuls can carry only one semaphore wait,
so operands must arrive via engine sems; DVE copies also perform the
f32->fp32r rounding the hardware verifier requires.
"""
import numpy as np
from contextlib import ExitStack

import concourse.bass as bass
import concourse.bacc as bacc
import concourse.tile as tile
from concourse import mybir
from concourse.bass_utils import run_bass_kernel_spmd

F32 = mybir.dt.float32
F32R = mybir.dt.float32r
AF = mybir.ActivationFunctionType
OP = mybir.AluOpType

B, S, H, NCORE = 256, 128, 512, 8
BL = B // NCORE          # 32
G4 = 4 * H               # 2048
NCH = H // 128           # 4


class Cfg:
    # 'f32' | 'f32r' | 'f32r_hsplit' for the LSTM/q matmul path
    lstm = "f32r_hsplit"
    # 'f32' | 'f32r' for the attention v-dot
    vdot = "f32"
    vsplit = False        # v-dot with v split into hi+lo (2 passes)


def round_fp32r(x):
    u = np.asarray(x, np.float32).view(np.uint32).copy()
    low12 = u & np.uint32(0xFFF)
    base = u & ~np.uint32(0xFFF)
    lsb = (u >> np.uint32(12)) & np.uint32(1)
    up = (low12 > 0x800) | ((low12 == 0x800) & (lsb == 1))
    return (base + np.where(up, np.uint32(0x1000), np.uint32(0))).view(
        np.float32).reshape(np.shape(x))


def build_nc(nc, cfg, enc_steps=S, dec_steps=S):
    hsplit = cfg.lstm == "f32r_hsplit"
    lstm_dt = F32 if cfg.lstm == "f32" else F32R
    vdot_dt = F32 if cfg.vdot == "f32" else F32R
    nvp = 2 if cfg.vsplit else 1
    nh = 2 if hsplit else 1

    def din(name, shape, dt=F32):
        return nc.dram_tensor(name, shape, dt, kind="ExternalInput").ap()

    x_b_d = din("x_b", (BL, S))
    xe_e_d = din("xe_e", (2, S * BL))
    wb_e_d = din("wb_e", (2, G4))
    whh_e_d = din("whh_e", (H, G4))
    wb_d_d = din("wb_d", (2, G4))
    whh_d_d = din("whh_d", (H, G4))
    w1t_d = din("w1t", (H, H))
    b1_d = din("b1r", (1, H))
    w2t_d = din("w2t", (H, H))
    b2_d = din("b2r", (1, H))
    vl_d = din("vl", (H, 64 * nvp))
    iota_d = din("iota", (BL, S))
    i32_d = din("i32", (32, 32))

    scores_d = nc.dram_tensor("scores", (BL, S * S), F32,
                              kind="ExternalOutput").ap()
    ptr_d = nc.dram_tensor("ptr", (BL, S), mybir.dt.int32,
                           kind="ExternalOutput").ap()

    with tile.TileContext(nc) as tc, ExitStack() as ctx:
        consts = ctx.enter_context(tc.tile_pool(name="consts", bufs=1))
        state = ctx.enter_context(tc.tile_pool(name="state", bufs=1))
        hpool = ctx.enter_context(tc.tile_pool(name="hpool", bufs=2))
        work = ctx.enter_context(tc.tile_pool(name="work", bufs=1))
        dram = ctx.enter_context(tc.tile_pool(name="dram", bufs=1,
                                              space="DRAM"))

        u_dram = dram.tile([BL, S, S], F32)

        def stage(pool, name, dram_ap, shape, dt, rearr=None, chunk=None):
            """DMA into staging tile(s), DVE-copy (rounds) into final tile.
            Keeps matmul operands producible by DVE (single-wait rule)."""
            t = pool.tile(shape, dt, name=name)
            src = dram_ap.rearrange(rearr, p=128) if rearr else dram_ap
            if chunk is None:
                stg = pool.tile(shape, F32, name=f"stg_{name}",
                                tag="dma_stage", bufs=2)
                nc.sync.dma_start(out=stg, in_=src)
                nc.vector.tensor_copy(t, stg)
            else:
                n = shape[1] // chunk
                for c in range(n):
                    stg = pool.tile([shape[0], chunk], F32,
                                    name=f"stg_{name}_{c}", tag="dma_stage",
                                    bufs=2)
                    nc.sync.dma_start(out=stg,
                                      in_=src[:, c * chunk:(c + 1) * chunk])
                    nc.vector.tensor_copy(t[:, c * chunk:(c + 1) * chunk], stg)
            return t

        # small resident constants
        x_b = stage(consts, "x_b", x_b_d, [BL, S], F32)
        iota = stage(consts, "iota", iota_d, [BL, S], F32)
        i32 = stage(consts, "i32", i32_d, [32, 32], F32)
        vl = stage(consts, "vl", vl_d, [128, NCH, 64 * nvp], vdot_dt,
                   "(c p) m -> p c m")

        # state
        c_b = state.tile([BL, H], F32)
        nc.vector.memset(c_b, 0.0)
        hT_zero = state.tile([128, NCH, nh, BL], lstm_dt)
        nc.vector.memset(hT_zero.bitcast(F32), 0.0)
        xe_d = state.tile([2, BL], lstm_dt)
        nc.vector.memset(xe_d.bitcast(F32), 1.0)           # row1 stays all-ones
        nc.vector.memset(xe_d.bitcast(F32)[0:1, :], 0.0)   # x_0 = 0
        ones32 = state.tile([1, BL], F32)
        nc.vector.memset(ones32, 1.0)
        ones_l = state.tile([1, BL], lstm_dt)
        nc.vector.memset(ones_l.bitcast(F32), 1.0)
        m_acc = state.tile([BL, S], F32)
        ptr_acc = state.tile([BL, S], mybir.dt.int32)

        # w1enc stays resident through the whole kernel (64KB/partition)
        w1enc = state.tile([128, NCH, BL, S], F32)
        if enc_steps < S or dec_steps < S:
            nc.vector.memset(w1enc, 0.0)
            nc.vector.memset(m_acc, 0.0)
            nc.vector.memset(ptr_acc, 0)
            zz = work.tile([BL, S], F32, tag="scrz")
            nc.vector.memset(zz, 0.0)
            for tt in range(S):
                nc.sync.dma_start(out=u_dram[:, tt, :], in_=zz)

        def lstm_cell(gates_ps):
            s_if = work.tile([BL, 2 * H], F32, tag="sif")
            nc.scalar.activation(s_if, gates_ps[:, 0:2 * H], AF.Sigmoid)
            tg = work.tile([BL, H], F32, tag="tg")
            nc.scalar.activation(tg, gates_ps[:, 2 * H:3 * H], AF.Tanh)
            so = work.tile([BL, H], F32, tag="so")
            nc.scalar.activation(so, gates_ps[:, 3 * H:4 * H], AF.Sigmoid)
            t1 = work.tile([BL, H], F32, tag="t1")
            nc.vector.scalar_tensor_tensor(out=t1, in0=s_if[:, H:2 * H],
                                           scalar=0.0, in1=c_b,
                                           op0=OP.bypass, op1=OP.mult)
            t2 = work.tile([BL, H], F32, tag="t2")
            nc.vector.scalar_tensor_tensor(out=t2, in0=s_if[:, 0:H], scalar=0.0,
                                           in1=tg, op0=OP.bypass, op1=OP.mult)
            nc.vector.scalar_tensor_tensor(out=c_b, in0=t1, scalar=0.0, in1=t2,
                                           op0=OP.bypass, op1=OP.add)
            tc_t = work.tile([BL, H], F32, tag="tct")
            nc.scalar.activation(tc_t, c_b, AF.Tanh)
            h_b = work.tile([BL, H], F32, tag="hb")
            nc.vector.scalar_tensor_tensor(out=h_b, in0=so, scalar=0.0,
                                           in1=tc_t, op0=OP.bypass, op1=OP.mult)
            return h_b

        def transpose_h(h_b, psum_pool, dst, dst_f32=None):
            for c in range(NCH):
                tp = psum_pool.tile([128, 32], F32, tag="tp")
                nc.tensor.transpose(tp, h_b[:, c * 128:(c + 1) * 128], i32)
                nc.vector.tensor_copy(dst[:, c, 0, :], tp)
                if hsplit:
                    nc.vector.scalar_tensor_tensor(
                        out=dst[:, c, 1, :], in0=tp, scalar=0.0,
                        in1=dst[:, c, 0, :], op0=OP.bypass, op1=OP.subtract)
                if dst_f32 is not None:
                    nc.vector.tensor_copy(dst_f32[:, c, :], tp)

        def gates_mm(gates, xe_t, wb_sb, whh_sb, hT_prev):
            for n in range(4):
                nsl = slice(n * H, (n + 1) * H)
                for c in range(NCH):
                    for p in range(nh):
                        nc.tensor.matmul(gates[:, nsl], hT_prev[:, c, p, :],
                                         whh_sb[:, c, nsl],
                                         start=(c == 0 and p == 0), stop=False)
                nc.tensor.matmul(gates[:, nsl], xe_t, wb_sb[:, nsl],
                                 start=False, stop=True)

        # ============ encoder ============
        with tc.tile_pool(name="encc", bufs=1) as encc, \
             tc.tile_pool(name="eps", bufs=1, space="PSUM") as eps:
            whh_e_sb = encc.tile([128, NCH, G4], lstm_dt)
            for c in range(NCH):
                for hh in range(2):
                    stg = encc.tile([128, G4 // 2], F32,
                                    name=f"stgwe_{c}_{hh}",
                                    tag="dma_stage", bufs=2)
                    nc.sync.dma_start(
                        out=stg,
                        in_=whh_e_d.rearrange("(c p) m -> p c m", p=128)
                        [:, c, hh * G4 // 2:(hh + 1) * G4 // 2])
                    nc.vector.tensor_copy(
                        whh_e_sb[:, c, hh * G4 // 2:(hh + 1) * G4 // 2], stg)
            wb_e_sb = stage(encc, "wb_e", wb_e_d, [2, G4], lstm_dt, chunk=1024)
            xe_e = stage(encc, "xe_e", xe_e_d, [2, S * BL], lstm_dt,
                         chunk=2048)
            w1t = stage(encc, "w1t", w1t_d, [128, NCH, H], F32,
                        "(c p) m -> p c m")
            b1r = stage(encc, "b1r", b1_d, [1, H], F32)

            hT_prev = hT_zero
            for t in range(enc_steps):
                gates = eps.tile([BL, G4], F32, tag="gates")
                gates_mm(gates, xe_e[:, t * BL:(t + 1) * BL], wb_e_sb,
                         whh_e_sb, hT_prev)
                h_b = lstm_cell(gates)
                hT_t = hpool.tile([128, NCH, nh, BL], lstm_dt, tag="hT")
                hTf_t = (hpool.tile([128, NCH, BL], F32, tag="hTf",
                                     name="hTf_t")
                         if lstm_dt != F32 else None)
                transpose_h(h_b, eps, hT_t, hTf_t)
                if hTf_t is None:
                    hTf_t = hT_t.rearrange("p c o b -> p (c o) b")
                for cp in range(NCH):
                    w1p = eps.tile([128, 32], F32, tag="w1p")
                    csl = slice(cp * 128, (cp + 1) * 128)
                    for c in range(NCH):
                        nc.tensor.matmul(w1p, w1t[:, c, csl], hTf_t[:, c, :],
                                         start=(c == 0), stop=False)
                    nc.tensor.matmul(w1p, b1r[:, csl], ones32,
                                     start=False, stop=True)
                    nc.vector.tensor_copy(w1enc[:, cp, :, t], w1p)
                hT_prev = hT_t

        # ============ decoder ============
        with tc.tile_pool(name="decc", bufs=1) as decc, \
             tc.tile_pool(name="dps", bufs=1, space="PSUM") as dps:
            whh_d_sb = decc.tile([128, NCH, G4], lstm_dt)
            for c in range(NCH):
                for hh in range(2):
                    stg = decc.tile([128, G4 // 2], F32,
                                    name=f"stgwd_{c}_{hh}",
                                    tag="dma_stage", bufs=2)
                    nc.sync.dma_start(
                        out=stg,
                        in_=whh_d_d.rearrange("(c p) m -> p c m", p=128)
                        [:, c, hh * G4 // 2:(hh + 1) * G4 // 2])
                    nc.vector.tensor_copy(
                        whh_d_sb[:, c, hh * G4 // 2:(hh + 1) * G4 // 2], stg)
            wb_d_sb = stage(decc, "wb_d", wb_d_d, [2, G4], lstm_dt, chunk=1024)
            w2t = decc.tile([128, NCH, H], lstm_dt)
            for c in range(NCH):
                stg = decc.tile([128, H], F32, name=f"stgw2_{c}",
                                tag="dma_stage", bufs=2)
                nc.sync.dma_start(
                    out=stg,
                    in_=w2t_d.rearrange("(c p) m -> p c m", p=128)[:, c, :])
                nc.vector.tensor_copy(w2t[:, c, :], stg)
            b2r = stage(decc, "b2r", b2_d, [1, H], lstm_dt)

            with tc.tile_pool(name="scrp", bufs=2) as scrp:
                for t in range(dec_steps):
                    gates = dps.tile([BL, G4], F32, tag="gates")
                    gates_mm(gates, xe_d, wb_d_sb, whh_d_sb, hT_prev)
                    h_b = lstm_cell(gates)
                    hT_t = hpool.tile([128, NCH, nh, BL], lstm_dt, tag="hT")
                    transpose_h(h_b, dps, hT_t)
                    hT_prev = hT_t

                    qps = dps.tile([BL, H], F32, tag="qps")
                    for c in range(NCH):
                        for p in range(nh):
                            nc.tensor.matmul(qps, hT_t[:, c, p, :],
                                             w2t[:, c, :],
                                             start=(c == 0 and p == 0),
                                             stop=False)
                    nc.tensor.matmul(qps, ones_l, b2r, start=False, stop=True)
                    q_sb = work.tile([BL, H], F32, tag="qsb")
                    nc.vector.tensor_copy(q_sb, qps)
                    qT = work.tile([128, NCH, BL], F32, tag="qT", bufs=2)
                    for c in range(NCH):
                        tp = dps.tile([128, 32], F32, tag="tp")
                        nc.tensor.transpose(tp, q_sb[:, c * 128:(c + 1) * 128],
                                            i32)
                        nc.vector.tensor_copy(qT[:, c, :], tp)

                    u8 = dps.tile([8, 512], F32, tag="u8")
                    nmm = NCH * 8 * nvp
                    imm = 0
                    for cp in range(NCH):
                        scr = scrp.tile([128, BL, S], F32, tag="scr")
                        qb = qT[:, cp, :]
                        q_bc = bass.AP(tensor=qb.tensor, offset=qb.offset,
                                       ap=[qb.ap[0], [1, BL], [0, S]])
                        nc.vector.scalar_tensor_tensor(
                            out=scr, in0=w1enc[:, cp, :, :], scalar=0.0,
                            in1=q_bc, op0=OP.bypass, op1=OP.add)
                        scr_t = (scr.bitcast(vdot_dt) if vdot_dt != F32
                                 else scr)
                        nc.scalar.activation(scr_t, scr, AF.Tanh)
                        scrf = scr_t.rearrange("p b s -> p (b s)")
                        for j in range(8):
                            for p in range(nvp):
                                jo = (j * nvp + p) * 8
                                nc.tensor.matmul(
                                    u8, vl[:, cp, jo:jo + 8],
                                    scrf[:, j * 512:(j + 1) * 512],
                                    start=(imm == 0), stop=(imm == nmm - 1))
                                imm += 1
                    u8_sb = work.tile([8, 512], F32, tag="u8sb", bufs=2)
                    nc.vector.tensor_copy(u8_sb, u8)
                    u32 = work.tile([BL, S], F32, tag="u32", bufs=2)
                    nc.sync.dma_start(
                        out=u32, in_=u8_sb.rearrange("p (b s) -> p b s", b=4))
                    nc.sync.dma_start(out=u_dram[:, t, :], in_=u32)

                    nc.vector.tensor_reduce(out=m_acc[:, t:t + 1], in_=u32,
                                            axis=mybir.AxisListType.X,
                                            op=OP.max)
                    mx8 = work.tile([BL, 8], F32, tag="mx8", bufs=2)
                    nc.vector.max(mx8, u32)
                    idx = work.tile([BL, 8], mybir.dt.uint32, tag="idx",
                                    bufs=2)
                    nc.vector.max_index(idx, mx8, u32)
                    nc.vector.tensor_copy(ptr_acc[:, t:t + 1], idx[:, 0:1])
                    idxf = work.tile([BL, 1], F32, tag="idxf", bufs=2)
                    nc.vector.tensor_copy(idxf, idx[:, 0:1])
                    msk = work.tile([BL, S], F32, tag="msk", bufs=2)
                    nc.vector.scalar_tensor_tensor(out=msk, in0=iota,
                                                   scalar=idxf, in1=x_b,
                                                   op0=OP.is_equal,
                                                   op1=OP.mult)
                    x_next = work.tile([BL, 1], F32, tag="xnext", bufs=2)
                    nc.vector.tensor_reduce(out=x_next, in_=msk,
                                            axis=mybir.AxisListType.X,
                                            op=OP.add)
                    tp = dps.tile([1, 32], F32, tag="tp")
                    nc.tensor.transpose(tp, x_next, i32)
                    nc.vector.tensor_copy(xe_d[0:1, :], tp)

                # ===== post: log-softmax normalization =====
                TB = 32
                z_sb = work.tile([BL, S], F32, tag="zsb")
                for tb in range(S // TB):
                    tsl = slice(tb * TB, (tb + 1) * TB)
                    ub = scrp.tile([BL, TB, S], F32, tag="scr")
                    nc.sync.dma_start(out=ub, in_=u_dram[:, tsl, :])
                    mb = m_acc[:, tsl]
                    m_bc = bass.AP(tensor=mb.tensor, offset=mb.offset,
                                   ap=[mb.ap[0], [1, TB], [0, S]])
                    nc.vector.scalar_tensor_tensor(out=ub, in0=ub, scalar=0.0,
                                                   in1=m_bc, op0=OP.bypass,
                                                   op1=OP.subtract)
                    nc.scalar.activation(ub, ub, AF.Exp)
                    nc.vector.tensor_reduce(out=z_sb[:, tsl], in_=ub,
                                            axis=mybir.AxisListType.X,
                                            op=OP.add)
                lnz = work.tile([BL, S], F32, tag="lnz")
                nc.scalar.activation(lnz, z_sb, AF.Ln)
                mzl = work.tile([BL, S], F32, tag="mzl")
                nc.vector.scalar_tensor_tensor(out=mzl, in0=m_acc, scalar=0.0,
                                               in1=lnz, op0=OP.bypass,
                                               op1=OP.add)
                for tb in range(S // TB):
                    tsl = slice(tb * TB, (tb + 1) * TB)
                    blk = scrp.tile([BL, TB, S], F32, tag="scr")
                    nc.sync.dma_start(out=blk, in_=u_dram[:, tsl, :])
                    mm_ = mzl[:, tsl]
                    mz_bc = bass.AP(tensor=mm_.tensor, offset=mm_.offset,
                                    ap=[mm_.ap[0], [1, TB], [0, S]])
                    nc.vector.scalar_tensor_tensor(out=blk, in0=blk,
                                                   scalar=0.0, in1=mz_bc,
                                                   op0=OP.bypass,
                                                   op1=OP.subtract)
                    nc.sync.dma_start(
                        out=scores_d[:, tb * TB * S:(tb + 1) * TB * S],
                        in_=blk.rearrange("p a s -> p (a s)"))
                nc.sync.dma_start(out=ptr_d, in_=ptr_acc)

    return nc


def make_inputs(inputs, cfg):
    f = np.float32
    Wih_e, We_e = inputs["Wih_e"].astype(f), inputs["We_e"].astype(f)
    Wih_d, We_d = inputs["Wih_d"].astype(f), inputs["We_d"].astype(f)
    weff_e = (Wih_e @ We_e)[:, 0]
    beff_e = (inputs["b_e"] + Wih_e @ inputs["be_e"]).astype(f)
    weff_d = (Wih_d @ We_d)[:, 0]
    beff_d = (inputs["b_d"] + Wih_d @ inputs["be_d"]).astype(f)
    wb_e = np.stack([weff_e, beff_e]).astype(f)
    wb_d = np.stack([weff_d, beff_d]).astype(f)
    whh_e = np.ascontiguousarray(inputs["Whh_e"].T).astype(f)
    whh_d = np.ascontiguousarray(inputs["Whh_d"].T).astype(f)
    w1t = np.ascontiguousarray(inputs["W1"].T).astype(f)
    w2t = np.ascontiguousarray(inputs["W2"].T).astype(f)
    b1r = inputs["b1"][None, :].astype(f)
    b2r = inputs["b2"][None, :].astype(f)
    v = inputs["v"][0].astype(f)
    bv = float(inputs["bv"][0])
    nvp = 2 if cfg.vsplit else 1
    vl = np.zeros((H, 64 * nvp), f)
    if cfg.vsplit:
        v_hi = round_fp32r(v)
        v_lo = (v - v_hi).astype(f)
        for j in range(8):
            vl[:, (j * 2) * 8 + j] = v_hi
            vl[:, (j * 2 + 1) * 8 + j] = v_lo
    else:
        for j in range(8):
            vl[:, j * 8 + j] = v
    iota = np.tile(np.arange(S, dtype=f), (BL, 1))
    i32 = np.eye(32, dtype=f)
    x_all = inputs["input_seq"][:, :, 0].astype(f)

    shared = dict(wb_e=wb_e, whh_e=whh_e, wb_d=wb_d, whh_d=whh_d,
                  w1t=w1t, b1r=b1r, w2t=w2t, b2r=b2r, vl=vl,
                  iota=iota, i32=i32)
    maps = []
    for cid in range(NCORE):
        xc = x_all[cid * BL:(cid + 1) * BL]
        xe_e = np.stack([np.ascontiguousarray(xc.T).reshape(-1),
                         np.ones(S * BL, f)]).astype(f)
        maps.append(dict(shared, x_b=xc.copy(), xe_e=xe_e))
    return maps, bv


def run(inputs, cfg=None, trace=False, trace_cores=None):
    cfg = cfg or Cfg()
    maps, bv = make_inputs(inputs, cfg)
    nc = bacc.Bacc("TRN2", target_bir_lowering=False, debug=False)
    nc = build_nc(nc, cfg)
    nc.compile()
    res = run_bass_kernel_spmd(nc, maps, core_ids=list(range(NCORE)),
                               trace=trace, trace_cores=trace_cores)
    scores = np.concatenate(
        [r["scores"].reshape(BL, S, S) for r in res.results], axis=0)
    ptrs = np.concatenate([r["ptr"] for r in res.results], axis=0)
    return scores, ptrs, res


def kernel(**inputs):
    scores, ptrs, _ = run(inputs)
    return scores, ptrs
